# revision 29
# baseline (speedup 1.0000x reference)
"""Trainium2 Bass kernel for nn_KMeansPalettizedLinear.

Computes y = x @ (lut[weight_idx])^T + bias for
  x: [4, 2048, 4096] f32, lut: [256] f32, weight_idx: [4096, 4096] i32,
  bias: [4096] f32  ->  y: [4, 2048, 4096] f32.

Strategy (column/tensor-parallel across 8 NeuronCores):
  - Host: dequantize W = lut[weight_idx] (palette gather), transpose X to
    X^T [D_IN, M], shard W^T/bias along out_features (512 per core).
  - Device (per core): Y_shard[m, o] = sum_d X^T[d, m] * W^T[d, o] + bias[o]
    as a tiled PE matmul with the X^T tile as the stationary operand
    (lhsT [128d, 128m]) and the SBUF-resident W^T as the moving operand
    ([128d, 512o]), accumulating over the 32 k-tiles in PSUM.
  - Matmul dtype is fp16 by default (PE upconverts to FP22 internally;
    ~1e-4 relative error) at full 1 cycle/row throughput.  fp8 was
    evaluated and rejected: e4m3 quantization gives 4.1e-2 max-rel error
    (gate is 2e-2), even one fp8 operand gives 2.7e-2.
  - Queue discipline: the x stream owns the SP HWDGE ring; W chunks, bias
    and the compute-gated y output DMAs go on the ACT ring so they never
    block x-tile prefetch (in-order ring: a waiting output DMA would stall
    every input DMA queued behind it).
  - W is loaded in per-ko chunks so mg0's matmuls start after 1/32 of W;
    a few junk warmup matmuls cover the first-DMA latency and start the
    PE HAM un-throttle window early.
"""

import os
import sys

sys.path.insert(0, "/opt/trn_rl_repo")

import numpy as np

B, S, D_IN, D_OUT, PALETTE = 4, 2048, 4096, 4096, 256
N_CORES = 8
M = B * S  # 8192
O_SHARD = D_OUT // N_CORES  # 512
P = 128
KO = D_IN // P  # 32 k-tiles
MG = M // 512  # 16 m-groups of 512 rows

# fp16 | bf16 | fp32r  (matmul input dtype; see module docstring)
MM_DTYPE = os.environ.get("KMEANS_MM_DTYPE", "fp16")
# >1 wraps the body in a device-side repeat loop (timing aid only)
REPEATS = int(os.environ.get("KMEANS_REPEATS", "1"))
# component isolation for timing experiments: all | xdma | mm | noepi
ONLY = os.environ.get("KMEANS_ONLY", "all")
# sharding strategy: o = column-parallel (x streamed), m = row-parallel
# (x resident, W streamed)
SHARD = os.environ.get("KMEANS_SHARD", "o")
M_SH = M // N_CORES  # 1024 rows per core (m-sharding)
OT = D_OUT // P  # 32 o-tiles (m-sharding)
# junk matmuls at kernel start: keeps PE busy while the first input DMAs
# land (~2us) and starts the HAM un-throttle window early.  Kept small —
# at the cold 1.2GHz clock each N=512 matmul is ~427ns, and overshooting
# the data-wait would delay the real stream.
WARM = int(os.environ.get("KMEANS_WARM", "4"))
# o-shard x DMA granularity: small = [128,512] per (mg,ko); big = [128,KO,512]
# per mg (4.2MB contiguous-ish, fewer ring transactions)
XMODE = os.environ.get("KMEANS_XMODE", "small")
# matmul issue order within an m-group: komi = rotate psum banks every MM;
# miko = 32 back-to-back accumulations per bank (needs XMODE=big or ONLY=mm)
XORDER = os.environ.get("KMEANS_XORDER", "komi")
# per-ko W-load split: W now streams on the ACT ring (x owns the SP ring),
# so the split no longer contends with x tiles and the first matmuls start
# after 1/32 of W instead of waiting out the full 4.2MB prologue
W_SPLIT = os.environ.get("KMEANS_W_SPLIT", "1") == "1"
# which HWDGE ring carries the y output DMAs: act keeps them off the
# x-stream ring
YQ = os.environ.get("KMEANS_YQ", "act")
# x DRAM layout: std = x^T [D_IN, M]; tiled = host pre-tiles to
# [(mg ko p), 512] so multi-ko x DMAs are possible.  Solo x-stream rate is
# per-DMA-overhead-bound: 512 DMAs/pass -> 195GB/s, 256 DMAs (XKO=2) ->
# 354GB/s (~HBM-per-NC limit), giving the stream 2.3x headroom over the
# 153.6GB/s the PE consumes.
XSRC = os.environ.get("KMEANS_XSRC", "tiled")
# k-tiles per x DMA (tiled layout only): 1 -> 131KB, 2 -> 262KB, 4 -> 524KB
XKO = int(os.environ.get("KMEANS_XKO", "2"))
X_BUFS = int(os.environ.get("KMEANS_X_BUFS", "12"))

_cache = {}


def _mm_dt():
    import concourse.mybir as mybir

    return {
        "fp16": (mybir.dt.float16, np.float16),
        "bf16": (mybir.dt.bfloat16, None),  # np side handled via ml_dtypes
        "fp32r": (mybir.dt.float32r, np.float32),
    }[MM_DTYPE]


def _np_cast(a):
    if MM_DTYPE == "fp16":
        return a.astype(np.float16)
    if MM_DTYPE == "bf16":
        import ml_dtypes

        return a.astype(ml_dtypes.bfloat16)
    return np.ascontiguousarray(a, dtype=np.float32)


def _emit_warmup(nc, warm_pool, pp, dt_mm):
    """Junk matmuls on a zeroed tile: occupy the PE through the HAM
    un-throttle window (~3.4us) while the first real DMAs land."""
    import concourse.mybir as mybir

    if WARM <= 0:
        return
    wx = warm_pool.tile([P, 512], dt_mm)
    nc.any.memset(wx[:], 0.0)
    wps = pp.tile([P, 512], mybir.dt.float32, tag="ps", name="warm_ps")
    for _ in range(WARM):
        nc.tensor.matmul(wps[:], wx[:, 0:P], wx[:], start=True, stop=True)


def _build():
    from concourse import bacc
    import concourse.mybir as mybir
    import concourse.tile as tile
    from concourse.bass import ds, ts

    dt_mm, _ = _mm_dt()
    nc = bacc.Bacc(None, target_bir_lowering=False)
    if XSRC == "tiled":
        xt = nc.dram_tensor("xt", [MG * KO * P, 512], dt_mm, kind="ExternalInput")
    else:
        xt = nc.dram_tensor("xt", [D_IN, M], dt_mm, kind="ExternalInput")
    wt = nc.dram_tensor("wt", [D_IN, O_SHARD], dt_mm, kind="ExternalInput")
    biasb = nc.dram_tensor("biasb", [P, O_SHARD], mybir.dt.float32, kind="ExternalInput")
    y = nc.dram_tensor("y", [M, O_SHARD], mybir.dt.float32, kind="ExternalOutput")

    with tile.TileContext(nc) as tc:
        with (
            tc.tile_pool(name="wpool", bufs=1) as wpool,
            tc.tile_pool(name="xpool", bufs=(4 if XMODE == "big" else X_BUFS)) as xpool,
            tc.tile_pool(name="opool", bufs=8) as opool,
            tc.tile_pool(name="cpool", bufs=1) as cpool,
            tc.tile_pool(name="warm", bufs=1) as warm_pool,
            tc.tile_pool(name="psum", bufs=8, space="PSUM") as pp,
        ):
            _emit_warmup(nc, warm_pool, pp, dt_mm)
            w_res = wpool.tile([P, KO, O_SHARD], dt_mm)
            wt_r = wt.rearrange("(ko p) o -> p ko o", p=P)
            # W on the ACT ring (x stream owns the SP ring); per-ko chunks
            # let mg0's matmuls start after 1/32 of W
            if W_SPLIT:
                for ko in range(KO):
                    nc.scalar.dma_start(w_res[:, ko, :], wt_r[:, ko, :])
            else:
                nc.scalar.dma_start(w_res[:], wt_r)
            bias_t = cpool.tile([P, O_SHARD], mybir.dt.float32)
            nc.scalar.dma_start(bias_t[:], biasb[:])

            import contextlib

            rep_ctx = (
                tc.For_i(0, REPEATS, 1) if REPEATS > 1 else contextlib.nullcontext()
            )
            with rep_ctx:
                _emit_body(nc, tc, xpool, opool, pp, w_res, bias_t, xt, y)
    nc.compile()
    return nc


def _emit_body(nc, tc, xpool, opool, pp, w_res, bias_t, xt, y):
    import concourse.mybir as mybir
    from concourse.bass import ds, ts

    dt_mm, _ = _mm_dt()
    if ONLY == "xdma":
        # x-stream only: measures achievable x DMA bandwidth
        for mg in range(MG):
            if XSRC == "tiled":
                for kk in range(KO // XKO):
                    xt_t = xpool.tile([P, XKO, 512], dt_mm, tag="xt")
                    base = (mg * KO + kk * XKO) * P
                    nc.sync.dma_start(
                        xt_t[:],
                        xt[ds(base, XKO * P), :].rearrange("(g p) m -> p g m", p=P),
                    )
            else:
                for ko in range(KO):
                    xt_t = xpool.tile([P, 512], dt_mm, tag="xt")
                    nc.sync.dma_start(xt_t[:], xt[ds(ko * P, P), ds(mg * 512, 512)])
        return
    if ONLY == "xdma2q":
        # x-stream split across both HWDGE queues (SP + Activation)
        for mg in range(MG):
            for ko in range(KO):
                xt_t = xpool.tile([P, 512], dt_mm, tag="xt")
                eng = nc.sync if ko % 2 == 0 else nc.scalar
                eng.dma_start(xt_t[:], xt[ds(ko * P, P), ds(mg * 512, 512)])
        return
    if ONLY == "xdmabig":
        # x-stream as one 4.2MB DMA per m-group (16 per iteration)
        xt_r = xt.rearrange("(ko p) m -> p ko m", p=P)
        for mg in range(MG):
            xt_t = xpool.tile([P, KO, 512], dt_mm, tag="xb")
            nc.sync.dma_start(xt_t[:], xt_r[:, :, ds(mg * 512, 512)])
        return
    x_static = None
    if ONLY == "mm":
        # PE-only: single memset x tile reused by every matmul
        x_static = xpool.tile([P, 512], dt_mm, tag="xs")
        nc.any.memset(x_static[:], 0.0)
    xt_r = xt.rearrange("(ko p) m -> p ko m", p=P)
    if True:
            for mg in range(MG):
                psums = [
                    pp.tile([P, O_SHARD], mybir.dt.float32, tag="ps", name=f"ps_{mg}_{i}")
                    for i in range(4)
                ]
                xt_big = None
                if XMODE == "big" and ONLY != "mm":
                    xt_big = xpool.tile([P, KO, 512], dt_mm, tag="xb")
                    if mg == 0:
                        # chunked: first matmuls start after 1/32 of the tile
                        for ko in range(KO):
                            nc.sync.dma_start(
                                xt_big[:, ko, :], xt_r[:, ko, ds(mg * 512, 512)]
                            )
                    else:
                        nc.sync.dma_start(xt_big[:], xt_r[:, :, ds(mg * 512, 512)])
                if ONLY != "mm" and XMODE != "big" and XSRC == "tiled":
                    assert XORDER == "komi", "miko needs XMODE=big"
                    for kk in range(KO // XKO):
                        t = xpool.tile([P, XKO, 512], dt_mm, tag="xt")
                        base = (mg * KO + kk * XKO) * P
                        nc.sync.dma_start(
                            t[:],
                            xt[ds(base, XKO * P), :].rearrange(
                                "(g p) m -> p g m", p=P
                            ),
                        )
                        for j in range(XKO):
                            ko = kk * XKO + j
                            for mi in range(4):
                                nc.tensor.matmul(
                                    psums[mi][:], t[:, j, ts(mi, P)],
                                    w_res[:, ko, :],
                                    start=(ko == 0), stop=(ko == KO - 1),
                                )
                elif ONLY != "mm" and XMODE != "big":
                    assert XORDER == "komi", "miko needs XMODE=big"
                    for ko in range(KO):
                        t = xpool.tile([P, 512], dt_mm, tag="xt")
                        nc.sync.dma_start(t[:], xt[ds(ko * P, P), ds(mg * 512, 512)])
                        for mi in range(4):
                            nc.tensor.matmul(
                                psums[mi][:], t[:, ts(mi, P)], w_res[:, ko, :],
                                start=(ko == 0), stop=(ko == KO - 1),
                            )
                else:
                    if XORDER == "miko":
                        order = [(ko, mi) for mi in range(4) for ko in range(KO)]
                    else:
                        order = [(ko, mi) for ko in range(KO) for mi in range(4)]
                    for ko, mi in order:
                        lhsT = (
                            x_static[:, ts(mi, P)] if ONLY == "mm"
                            else xt_big[:, ko, ts(mi, P)]
                        )
                        nc.tensor.matmul(
                            psums[mi][:], lhsT, w_res[:, ko, :],
                            start=(ko == 0), stop=(ko == KO - 1),
                        )
                if ONLY in ("noepi", "mm"):
                    continue
                for mi in range(4):
                    ot = opool.tile([P, O_SHARD], mybir.dt.float32, tag="ot")
                    nc.vector.tensor_tensor(
                        ot[:], psums[mi][:], bias_t[:], mybir.AluOpType.add
                    )
                    # y out on the ACT HWDGE queue: keeps the compute-gated
                    # output DMA from blocking the x-stream prefetch on SP
                    yeng = nc.scalar if YQ == "act" else nc.sync
                    yeng.dma_start(y[ds(mg * 512 + mi * P, P), :], ot[:])


def _build_ms():
    """Row-parallel (M-sharded) build: x^T resident in SBUF, W^T streamed.

    Per core: Y_c[1024m, 4096o] = X_c[1024, 4096] @ W^T + bias.
    Loop over 32 o-tiles; per o-tile stream W^T chunk [4096k, 128o] (1 MB,
    contiguous), matmul against resident x^T with W-ktile stationary
    (reused for 2 m-groups), psum [128o, 512m], ACT adds per-partition
    bias, DMA out y^T-tiled.
    """
    from concourse import bacc
    import concourse.mybir as mybir
    import concourse.tile as tile
    from concourse.bass import ds, ts

    dt_mm, _ = _mm_dt()
    nc = bacc.Bacc(None, target_bir_lowering=False)
    # xq[ko*128+p, m] = x_c[m, ko*128+p]  (= x_c.T, contiguous)
    xq = nc.dram_tensor("xq", [D_IN, M_SH], dt_mm, kind="ExternalInput")
    # wq[ot*128+p, ko*128+o] = W^T[ko*128+p, ot*128+o]
    wq = nc.dram_tensor("wq", [OT * P, KO * P], dt_mm, kind="ExternalInput")
    # bq[p, ot] = bias[ot*128+p]
    bq = nc.dram_tensor("bq", [P, OT], mybir.dt.float32, kind="ExternalInput")
    # yq[(ot*2+mgr)*128+p, m] = y_c[mgr*512+m, ot*128+p]
    y = nc.dram_tensor("y", [OT * 2 * P, 512], mybir.dt.float32, kind="ExternalOutput")

    with tile.TileContext(nc) as tc:
        with (
            tc.tile_pool(name="xres", bufs=1) as xrp,
            tc.tile_pool(name="wpool", bufs=4) as wpool,
            tc.tile_pool(name="opool", bufs=6) as opool,
            tc.tile_pool(name="cpool", bufs=1) as cpool,
            tc.tile_pool(name="psum", bufs=8, space="PSUM") as pp,
        ):
            bias_sb = cpool.tile([P, OT], mybir.dt.float32)
            nc.sync.dma_start(bias_sb[:], bq[:])
            x_res = xrp.tile([P, KO, M_SH], dt_mm)
            for ko in range(KO):
                nc.sync.dma_start(x_res[:, ko, :], xq[ds(ko * P, P), :])

            import contextlib

            rep_ctx = (
                tc.For_i(0, REPEATS, 1) if REPEATS > 1 else contextlib.nullcontext()
            )
            with rep_ctx:
                for ot in range(OT):
                    w_t = wpool.tile([P, KO, P], dt_mm, tag="wt")
                    nc.sync.dma_start(
                        w_t[:], wq[ds(ot * P, P), :].rearrange("p (ko o) -> p ko o", o=P)
                    )
                    psums = [
                        pp.tile([P, 512], mybir.dt.float32, tag="ps", name=f"ps_{ot}_{i}")
                        for i in range(2)
                    ]
                    for ko in range(KO):
                        for mgr in range(2):
                            nc.tensor.matmul(
                                psums[mgr][:],
                                w_t[:, ko, :],
                                x_res[:, ko, ds(mgr * 512, 512)],
                                start=(ko == 0),
                                stop=(ko == KO - 1),
                            )
                    for mgr in range(2):
                        osb = opool.tile([P, 512], mybir.dt.float32, tag="ot")
                        nc.scalar.add(osb[:], psums[mgr][:], bias_sb[:, ds(ot, 1)])
                        # y out on ACT queue: SP carries only the W stream, so
                        # w(ot+1) issues as soon as its WAR clears instead of
                        # queueing behind this compute-gated output DMA
                        nc.scalar.dma_start(y[ds((ot * 2 + mgr) * P, P), :], osb[:])
    nc.compile()
    return nc


def make_in_maps_ms(input, lookup_table, weight_idx, bias):
    x = np.asarray(input, dtype=np.float32).reshape(M, D_IN)
    lut = np.asarray(lookup_table, dtype=np.float32)
    idx = np.asarray(weight_idx)
    b = np.asarray(bias, dtype=np.float32)

    wt_full = _np_cast(lut[idx].T)  # [D_IN, D_OUT]
    # wq[ot*128+p, ko*128+o] = wt_full[ko*128+p, ot*128+o]
    wq = np.ascontiguousarray(
        wt_full.reshape(KO, P, OT, P).transpose(2, 1, 0, 3).reshape(OT * P, KO * P)
    )
    bq = np.ascontiguousarray(b.reshape(OT, P).T)  # [P, OT]

    in_maps = []
    for c in range(N_CORES):
        xc = x[c * M_SH : (c + 1) * M_SH]  # [1024, 4096]
        xq = np.ascontiguousarray(_np_cast(xc).T)  # [D_IN, M_SH]
        in_maps.append({"xq": xq, "wq": wq, "bq": bq})
    return in_maps


def gather_ms(results):
    ys = []
    for c in range(N_CORES):
        yq = results[c]["y"].reshape(OT, 2, P, 512)
        ys.append(np.transpose(yq, (1, 3, 0, 2)).reshape(M_SH, D_OUT))
    return np.concatenate(ys, axis=0).reshape(B, S, D_OUT)


def get_nc():
    if "nc" not in _cache:
        _cache["nc"] = _build_ms() if SHARD == "m" else _build()
    return _cache["nc"]


def make_in_maps(input, lookup_table, weight_idx, bias):
    """Host-side shard/layout prep -> per-core input maps."""
    x = np.asarray(input, dtype=np.float32).reshape(M, D_IN)
    lut = np.asarray(lookup_table, dtype=np.float32)
    idx = np.asarray(weight_idx)
    b = np.asarray(bias, dtype=np.float32)

    xT = _np_cast(x).T  # [D_IN, M]
    if XSRC == "tiled":
        # pre-tile so each (mg, ko) x tile is one contiguous 131KB DRAM chunk
        xt = np.ascontiguousarray(
            xT.reshape(KO, P, MG, 512).transpose(2, 0, 1, 3).reshape(MG * KO * P, 512)
        )
    else:
        xt = np.ascontiguousarray(xT)
    wt_full = lut[idx].T  # [D_IN, D_OUT] f32 (palette dequant on host)

    in_maps = []
    for c in range(N_CORES):
        sl = slice(c * O_SHARD, (c + 1) * O_SHARD)
        in_maps.append(
            {
                "xt": xt,
                "wt": np.ascontiguousarray(_np_cast(wt_full[:, sl])),
                "biasb": np.ascontiguousarray(
                    np.broadcast_to(b[sl], (P, O_SHARD)), dtype=np.float32
                ),
            }
        )
    return in_maps


def kernel(input, lookup_table, weight_idx, bias):
    from concourse.bass_utils import run_bass_kernel_spmd

    nc = get_nc()
    if SHARD == "m":
        in_maps = make_in_maps_ms(input, lookup_table, weight_idx, bias)
        res = run_bass_kernel_spmd(nc, in_maps, core_ids=list(range(N_CORES)))
        return gather_ms(res.results)
    in_maps = make_in_maps(input, lookup_table, weight_idx, bias)
    res = run_bass_kernel_spmd(nc, in_maps, core_ids=list(range(N_CORES)))
    y = np.concatenate([res.results[c]["y"] for c in range(N_CORES)], axis=1)
    return y.reshape(B, S, D_OUT)



# revision 30
# speedup vs baseline: 1.0428x; 1.0428x over previous
"""Trainium2 Bass kernel for nn_KMeansPalettizedLinear.

Computes y = x @ (lut[weight_idx])^T + bias for
  x: [4, 2048, 4096] f32, lut: [256] f32, weight_idx: [4096, 4096] i32,
  bias: [4096] f32  ->  y: [4, 2048, 4096] f32.

Strategy (column/tensor-parallel across 8 NeuronCores):
  - Host: dequantize W = lut[weight_idx] (palette gather), transpose X to
    X^T [D_IN, M], shard W^T/bias along out_features (512 per core).
  - Device (per core): Y_shard[m, o] = sum_d X^T[d, m] * W^T[d, o] + bias[o]
    as a tiled PE matmul with the X^T tile as the stationary operand
    (lhsT [128d, 128m]) and the SBUF-resident W^T as the moving operand
    ([128d, 512o]), accumulating over the 32 k-tiles in PSUM.
  - Matmul dtype is fp16 by default (PE upconverts to FP22 internally;
    ~1e-4 relative error) at full 1 cycle/row throughput.  fp8 was
    evaluated and rejected: e4m3 quantization gives 4.1e-2 max-rel error
    (gate is 2e-2), even one fp8 operand gives 2.7e-2.
  - Queue discipline: the x stream owns the SP HWDGE ring; W chunks, bias
    and the compute-gated y output DMAs go on the ACT ring so they never
    block x-tile prefetch (in-order ring: a waiting output DMA would stall
    every input DMA queued behind it).
  - W is loaded in per-ko chunks so mg0's matmuls start after 1/32 of W;
    a few junk warmup matmuls cover the first-DMA latency and start the
    PE HAM un-throttle window early.
"""

import os
import sys

sys.path.insert(0, "/opt/trn_rl_repo")

import numpy as np

B, S, D_IN, D_OUT, PALETTE = 4, 2048, 4096, 4096, 256
N_CORES = 8
M = B * S  # 8192
O_SHARD = D_OUT // N_CORES  # 512
P = 128
KO = D_IN // P  # 32 k-tiles
MG = M // 512  # 16 m-groups of 512 rows

# fp16 | bf16 | fp32r  (matmul input dtype; see module docstring)
MM_DTYPE = os.environ.get("KMEANS_MM_DTYPE", "fp16")
# >1 wraps the body in a device-side repeat loop (timing aid only)
REPEATS = int(os.environ.get("KMEANS_REPEATS", "1"))
# component isolation for timing experiments: all | xdma | mm | noepi
ONLY = os.environ.get("KMEANS_ONLY", "all")
# sharding strategy: o = column-parallel (x streamed), m = row-parallel
# (x resident, W streamed)
SHARD = os.environ.get("KMEANS_SHARD", "o")
M_SH = M // N_CORES  # 1024 rows per core (m-sharding)
OT = D_OUT // P  # 32 o-tiles (m-sharding)
# junk matmuls at kernel start: keeps PE busy while the first input DMAs
# land (~2us) and starts the HAM un-throttle window early.  Kept small —
# at the cold 1.2GHz clock each N=512 matmul is ~427ns, and overshooting
# the data-wait would delay the real stream.
WARM = int(os.environ.get("KMEANS_WARM", "4"))
# o-shard x DMA granularity: small = [128,512] per (mg,ko); big = [128,KO,512]
# per mg (4.2MB contiguous-ish, fewer ring transactions)
XMODE = os.environ.get("KMEANS_XMODE", "small")
# matmul issue order within an m-group: komi = rotate psum banks every MM;
# miko = 32 back-to-back accumulations per bank (needs XMODE=big or ONLY=mm)
XORDER = os.environ.get("KMEANS_XORDER", "komi")
# per-ko W-load split: W now streams on the ACT ring (x owns the SP ring),
# so the split no longer contends with x tiles and the first matmuls start
# after 1/32 of W instead of waiting out the full 4.2MB prologue
W_SPLIT = os.environ.get("KMEANS_W_SPLIT", "1") == "1"
# which HWDGE ring carries the y output DMAs: act keeps them off the
# x-stream ring
YQ = os.environ.get("KMEANS_YQ", "act")
# x DRAM layout: std = x^T [D_IN, M]; tiled = host pre-tiles to
# [(mg ko p), 512] so multi-ko x DMAs are possible.  Solo x-stream rate is
# per-DMA-overhead-bound: 512 DMAs/pass -> 195GB/s, 256 DMAs (XKO=2) ->
# 354GB/s (~HBM-per-NC limit), giving the stream 2.3x headroom over the
# 153.6GB/s the PE consumes.
XSRC = os.environ.get("KMEANS_XSRC", "tiled")
# k-tiles per x DMA (tiled layout only): 1 -> 131KB, 2 -> 262KB, 4 -> 524KB
XKO = int(os.environ.get("KMEANS_XKO", "2"))
X_BUFS = int(os.environ.get("KMEANS_X_BUFS", "12"))

_cache = {}


def _mm_dt():
    import concourse.mybir as mybir

    return {
        "fp16": (mybir.dt.float16, np.float16),
        "bf16": (mybir.dt.bfloat16, None),  # np side handled via ml_dtypes
        "fp32r": (mybir.dt.float32r, np.float32),
    }[MM_DTYPE]


def _np_cast(a):
    if MM_DTYPE == "fp16":
        return a.astype(np.float16)
    if MM_DTYPE == "bf16":
        import ml_dtypes

        return a.astype(ml_dtypes.bfloat16)
    return np.ascontiguousarray(a, dtype=np.float32)


def _emit_warmup(nc, warm_pool, pp, dt_mm):
    """Junk matmuls on a zeroed tile: occupy the PE through the HAM
    un-throttle window (~3.4us) while the first real DMAs land."""
    import concourse.mybir as mybir

    if WARM <= 0:
        return
    wx = warm_pool.tile([P, 512], dt_mm)
    nc.any.memset(wx[:], 0.0)
    wps = pp.tile([P, 512], mybir.dt.float32, tag="ps", name="warm_ps")
    for _ in range(WARM):
        nc.tensor.matmul(wps[:], wx[:, 0:P], wx[:], start=True, stop=True)


def _build():
    from concourse import bacc
    import concourse.mybir as mybir
    import concourse.tile as tile
    from concourse.bass import ds, ts

    dt_mm, _ = _mm_dt()
    nc = bacc.Bacc(None, target_bir_lowering=False)
    if XSRC == "tiled":
        xt = nc.dram_tensor("xt", [MG * KO * P, 512], dt_mm, kind="ExternalInput")
    else:
        xt = nc.dram_tensor("xt", [D_IN, M], dt_mm, kind="ExternalInput")
    wt = nc.dram_tensor("wt", [D_IN, O_SHARD], dt_mm, kind="ExternalInput")
    biasb = nc.dram_tensor("biasb", [P, O_SHARD], mybir.dt.float32, kind="ExternalInput")
    y = nc.dram_tensor("y", [M, O_SHARD], mybir.dt.float32, kind="ExternalOutput")

    with tile.TileContext(nc) as tc:
        with (
            tc.tile_pool(name="wpool", bufs=1) as wpool,
            tc.tile_pool(name="xpool", bufs=(4 if XMODE == "big" else X_BUFS)) as xpool,
            tc.tile_pool(name="opool", bufs=8) as opool,
            tc.tile_pool(name="cpool", bufs=1) as cpool,
            tc.tile_pool(name="warm", bufs=1) as warm_pool,
            tc.tile_pool(name="psum", bufs=8, space="PSUM") as pp,
        ):
            _emit_warmup(nc, warm_pool, pp, dt_mm)
            w_res = wpool.tile([P, KO, O_SHARD], dt_mm)
            wt_r = wt.rearrange("(ko p) o -> p ko o", p=P)
            # W on the ACT ring (x stream owns the SP ring); 2-ko chunks let
            # mg0's matmuls start after 1/16 of W while keeping per-DMA
            # overhead low enough to outpace mg0's consumption (154GB/s)
            if W_SPLIT:
                for kk in range(KO // 2):
                    nc.scalar.dma_start(
                        w_res[:, ds(2 * kk, 2), :], wt_r[:, ds(2 * kk, 2), :]
                    )
            else:
                nc.scalar.dma_start(w_res[:], wt_r)
            bias_t = cpool.tile([P, O_SHARD], mybir.dt.float32)
            nc.scalar.dma_start(bias_t[:], biasb[:])

            import contextlib

            rep_ctx = (
                tc.For_i(0, REPEATS, 1) if REPEATS > 1 else contextlib.nullcontext()
            )
            with rep_ctx:
                _emit_body(nc, tc, xpool, opool, pp, w_res, bias_t, xt, y)
    nc.compile()
    return nc


def _emit_body(nc, tc, xpool, opool, pp, w_res, bias_t, xt, y):
    import concourse.mybir as mybir
    from concourse.bass import ds, ts

    dt_mm, _ = _mm_dt()
    if ONLY == "xdma":
        # x-stream only: measures achievable x DMA bandwidth
        for mg in range(MG):
            if XSRC == "tiled":
                for kk in range(KO // XKO):
                    xt_t = xpool.tile([P, XKO, 512], dt_mm, tag="xt")
                    base = (mg * KO + kk * XKO) * P
                    nc.sync.dma_start(
                        xt_t[:],
                        xt[ds(base, XKO * P), :].rearrange("(g p) m -> p g m", p=P),
                    )
            else:
                for ko in range(KO):
                    xt_t = xpool.tile([P, 512], dt_mm, tag="xt")
                    nc.sync.dma_start(xt_t[:], xt[ds(ko * P, P), ds(mg * 512, 512)])
        return
    if ONLY == "xdma2q":
        # x-stream split across both HWDGE queues (SP + Activation)
        for mg in range(MG):
            for ko in range(KO):
                xt_t = xpool.tile([P, 512], dt_mm, tag="xt")
                eng = nc.sync if ko % 2 == 0 else nc.scalar
                eng.dma_start(xt_t[:], xt[ds(ko * P, P), ds(mg * 512, 512)])
        return
    if ONLY == "xdmabig":
        # x-stream as one 4.2MB DMA per m-group (16 per iteration)
        xt_r = xt.rearrange("(ko p) m -> p ko m", p=P)
        for mg in range(MG):
            xt_t = xpool.tile([P, KO, 512], dt_mm, tag="xb")
            nc.sync.dma_start(xt_t[:], xt_r[:, :, ds(mg * 512, 512)])
        return
    x_static = None
    if ONLY == "mm":
        # PE-only: single memset x tile reused by every matmul
        x_static = xpool.tile([P, 512], dt_mm, tag="xs")
        nc.any.memset(x_static[:], 0.0)
    xt_r = xt.rearrange("(ko p) m -> p ko m", p=P)
    if True:
            for mg in range(MG):
                psums = [
                    pp.tile([P, O_SHARD], mybir.dt.float32, tag="ps", name=f"ps_{mg}_{i}")
                    for i in range(4)
                ]
                xt_big = None
                if XMODE == "big" and ONLY != "mm":
                    xt_big = xpool.tile([P, KO, 512], dt_mm, tag="xb")
                    if mg == 0:
                        # chunked: first matmuls start after 1/32 of the tile
                        for ko in range(KO):
                            nc.sync.dma_start(
                                xt_big[:, ko, :], xt_r[:, ko, ds(mg * 512, 512)]
                            )
                    else:
                        nc.sync.dma_start(xt_big[:], xt_r[:, :, ds(mg * 512, 512)])
                if ONLY != "mm" and XMODE != "big" and XSRC == "tiled":
                    assert XORDER == "komi", "miko needs XMODE=big"
                    for kk in range(KO // XKO):
                        t = xpool.tile([P, XKO, 512], dt_mm, tag="xt")
                        base = (mg * KO + kk * XKO) * P
                        nc.sync.dma_start(
                            t[:],
                            xt[ds(base, XKO * P), :].rearrange(
                                "(g p) m -> p g m", p=P
                            ),
                        )
                        for j in range(XKO):
                            ko = kk * XKO + j
                            for mi in range(4):
                                nc.tensor.matmul(
                                    psums[mi][:], t[:, j, ts(mi, P)],
                                    w_res[:, ko, :],
                                    start=(ko == 0), stop=(ko == KO - 1),
                                )
                elif ONLY != "mm" and XMODE != "big":
                    assert XORDER == "komi", "miko needs XMODE=big"
                    for ko in range(KO):
                        t = xpool.tile([P, 512], dt_mm, tag="xt")
                        nc.sync.dma_start(t[:], xt[ds(ko * P, P), ds(mg * 512, 512)])
                        for mi in range(4):
                            nc.tensor.matmul(
                                psums[mi][:], t[:, ts(mi, P)], w_res[:, ko, :],
                                start=(ko == 0), stop=(ko == KO - 1),
                            )
                else:
                    if XORDER == "miko":
                        order = [(ko, mi) for mi in range(4) for ko in range(KO)]
                    else:
                        order = [(ko, mi) for ko in range(KO) for mi in range(4)]
                    for ko, mi in order:
                        lhsT = (
                            x_static[:, ts(mi, P)] if ONLY == "mm"
                            else xt_big[:, ko, ts(mi, P)]
                        )
                        nc.tensor.matmul(
                            psums[mi][:], lhsT, w_res[:, ko, :],
                            start=(ko == 0), stop=(ko == KO - 1),
                        )
                if ONLY in ("noepi", "mm"):
                    continue
                for mi in range(4):
                    ot = opool.tile([P, O_SHARD], mybir.dt.float32, tag="ot")
                    nc.vector.tensor_tensor(
                        ot[:], psums[mi][:], bias_t[:], mybir.AluOpType.add
                    )
                    # y out on the ACT HWDGE queue: keeps the compute-gated
                    # output DMA from blocking the x-stream prefetch on SP
                    yeng = nc.scalar if YQ == "act" else nc.sync
                    yeng.dma_start(y[ds(mg * 512 + mi * P, P), :], ot[:])


def _build_ms():
    """Row-parallel (M-sharded) build: x^T resident in SBUF, W^T streamed.

    Per core: Y_c[1024m, 4096o] = X_c[1024, 4096] @ W^T + bias.
    Loop over 32 o-tiles; per o-tile stream W^T chunk [4096k, 128o] (1 MB,
    contiguous), matmul against resident x^T with W-ktile stationary
    (reused for 2 m-groups), psum [128o, 512m], ACT adds per-partition
    bias, DMA out y^T-tiled.
    """
    from concourse import bacc
    import concourse.mybir as mybir
    import concourse.tile as tile
    from concourse.bass import ds, ts

    dt_mm, _ = _mm_dt()
    nc = bacc.Bacc(None, target_bir_lowering=False)
    # xq[ko*128+p, m] = x_c[m, ko*128+p]  (= x_c.T, contiguous)
    xq = nc.dram_tensor("xq", [D_IN, M_SH], dt_mm, kind="ExternalInput")
    # wq[ot*128+p, ko*128+o] = W^T[ko*128+p, ot*128+o]
    wq = nc.dram_tensor("wq", [OT * P, KO * P], dt_mm, kind="ExternalInput")
    # bq[p, ot] = bias[ot*128+p]
    bq = nc.dram_tensor("bq", [P, OT], mybir.dt.float32, kind="ExternalInput")
    # yq[(ot*2+mgr)*128+p, m] = y_c[mgr*512+m, ot*128+p]
    y = nc.dram_tensor("y", [OT * 2 * P, 512], mybir.dt.float32, kind="ExternalOutput")

    with tile.TileContext(nc) as tc:
        with (
            tc.tile_pool(name="xres", bufs=1) as xrp,
            tc.tile_pool(name="wpool", bufs=4) as wpool,
            tc.tile_pool(name="opool", bufs=6) as opool,
            tc.tile_pool(name="cpool", bufs=1) as cpool,
            tc.tile_pool(name="psum", bufs=8, space="PSUM") as pp,
        ):
            bias_sb = cpool.tile([P, OT], mybir.dt.float32)
            nc.sync.dma_start(bias_sb[:], bq[:])
            x_res = xrp.tile([P, KO, M_SH], dt_mm)
            for ko in range(KO):
                nc.sync.dma_start(x_res[:, ko, :], xq[ds(ko * P, P), :])

            import contextlib

            rep_ctx = (
                tc.For_i(0, REPEATS, 1) if REPEATS > 1 else contextlib.nullcontext()
            )
            with rep_ctx:
                for ot in range(OT):
                    w_t = wpool.tile([P, KO, P], dt_mm, tag="wt")
                    nc.sync.dma_start(
                        w_t[:], wq[ds(ot * P, P), :].rearrange("p (ko o) -> p ko o", o=P)
                    )
                    psums = [
                        pp.tile([P, 512], mybir.dt.float32, tag="ps", name=f"ps_{ot}_{i}")
                        for i in range(2)
                    ]
                    for ko in range(KO):
                        for mgr in range(2):
                            nc.tensor.matmul(
                                psums[mgr][:],
                                w_t[:, ko, :],
                                x_res[:, ko, ds(mgr * 512, 512)],
                                start=(ko == 0),
                                stop=(ko == KO - 1),
                            )
                    for mgr in range(2):
                        osb = opool.tile([P, 512], mybir.dt.float32, tag="ot")
                        nc.scalar.add(osb[:], psums[mgr][:], bias_sb[:, ds(ot, 1)])
                        # y out on ACT queue: SP carries only the W stream, so
                        # w(ot+1) issues as soon as its WAR clears instead of
                        # queueing behind this compute-gated output DMA
                        nc.scalar.dma_start(y[ds((ot * 2 + mgr) * P, P), :], osb[:])
    nc.compile()
    return nc


def make_in_maps_ms(input, lookup_table, weight_idx, bias):
    x = np.asarray(input, dtype=np.float32).reshape(M, D_IN)
    lut = np.asarray(lookup_table, dtype=np.float32)
    idx = np.asarray(weight_idx)
    b = np.asarray(bias, dtype=np.float32)

    wt_full = _np_cast(lut[idx].T)  # [D_IN, D_OUT]
    # wq[ot*128+p, ko*128+o] = wt_full[ko*128+p, ot*128+o]
    wq = np.ascontiguousarray(
        wt_full.reshape(KO, P, OT, P).transpose(2, 1, 0, 3).reshape(OT * P, KO * P)
    )
    bq = np.ascontiguousarray(b.reshape(OT, P).T)  # [P, OT]

    in_maps = []
    for c in range(N_CORES):
        xc = x[c * M_SH : (c + 1) * M_SH]  # [1024, 4096]
        xq = np.ascontiguousarray(_np_cast(xc).T)  # [D_IN, M_SH]
        in_maps.append({"xq": xq, "wq": wq, "bq": bq})
    return in_maps


def gather_ms(results):
    ys = []
    for c in range(N_CORES):
        yq = results[c]["y"].reshape(OT, 2, P, 512)
        ys.append(np.transpose(yq, (1, 3, 0, 2)).reshape(M_SH, D_OUT))
    return np.concatenate(ys, axis=0).reshape(B, S, D_OUT)


def get_nc():
    if "nc" not in _cache:
        _cache["nc"] = _build_ms() if SHARD == "m" else _build()
    return _cache["nc"]


def make_in_maps(input, lookup_table, weight_idx, bias):
    """Host-side shard/layout prep -> per-core input maps."""
    x = np.asarray(input, dtype=np.float32).reshape(M, D_IN)
    lut = np.asarray(lookup_table, dtype=np.float32)
    idx = np.asarray(weight_idx)
    b = np.asarray(bias, dtype=np.float32)

    xT = _np_cast(x).T  # [D_IN, M]
    if XSRC == "tiled":
        # pre-tile so each (mg, ko) x tile is one contiguous 131KB DRAM chunk
        xt = np.ascontiguousarray(
            xT.reshape(KO, P, MG, 512).transpose(2, 0, 1, 3).reshape(MG * KO * P, 512)
        )
    else:
        xt = np.ascontiguousarray(xT)
    wt_full = lut[idx].T  # [D_IN, D_OUT] f32 (palette dequant on host)

    in_maps = []
    for c in range(N_CORES):
        sl = slice(c * O_SHARD, (c + 1) * O_SHARD)
        in_maps.append(
            {
                "xt": xt,
                "wt": np.ascontiguousarray(_np_cast(wt_full[:, sl])),
                "biasb": np.ascontiguousarray(
                    np.broadcast_to(b[sl], (P, O_SHARD)), dtype=np.float32
                ),
            }
        )
    return in_maps


def kernel(input, lookup_table, weight_idx, bias):
    from concourse.bass_utils import run_bass_kernel_spmd

    nc = get_nc()
    if SHARD == "m":
        in_maps = make_in_maps_ms(input, lookup_table, weight_idx, bias)
        res = run_bass_kernel_spmd(nc, in_maps, core_ids=list(range(N_CORES)))
        return gather_ms(res.results)
    in_maps = make_in_maps(input, lookup_table, weight_idx, bias)
    res = run_bass_kernel_spmd(nc, in_maps, core_ids=list(range(N_CORES)))
    y = np.concatenate([res.results[c]["y"] for c in range(N_CORES)], axis=1)
    return y.reshape(B, S, D_OUT)



# revision 31
# speedup vs baseline: 1.1624x; 1.1147x over previous
"""Trainium2 Bass kernel for nn_KMeansPalettizedLinear.

Computes y = x @ (lut[weight_idx])^T + bias for
  x: [4, 2048, 4096] f32, lut: [256] f32, weight_idx: [4096, 4096] i32,
  bias: [4096] f32  ->  y: [4, 2048, 4096] f32.

Strategy (column/tensor-parallel across 8 NeuronCores):
  - Host: dequantize W = lut[weight_idx] (palette gather), transpose X to
    X^T [D_IN, M], shard W^T/bias along out_features (512 per core).
  - Device (per core): Y_shard[m, o] = sum_d X^T[d, m] * W^T[d, o] + bias[o]
    as a tiled PE matmul with the X^T tile as the stationary operand
    (lhsT [128d, 128m]) and the SBUF-resident W^T as the moving operand
    ([128d, 512o]), accumulating over the 32 k-tiles in PSUM.
  - Matmul dtype is fp16 by default (PE upconverts to FP22 internally;
    ~1e-4 relative error) at full 1 cycle/row throughput.  fp8 was
    evaluated and rejected: e4m3 quantization gives 4.1e-2 max-rel error
    (gate is 2e-2), even one fp8 operand gives 2.7e-2.
  - Queue discipline: the x stream owns the SP HWDGE ring; W chunks, bias
    and the compute-gated y output DMAs go on the ACT ring so they never
    block x-tile prefetch (in-order ring: a waiting output DMA would stall
    every input DMA queued behind it).
  - W is loaded in 2-ko chunks so mg0's matmuls start after 1/16 of W;
    a few junk warmup matmuls cover the first-DMA latency and start the
    PE HAM un-throttle window early.
  - x streams as host-pre-tiled 2-ko [128, 2, 512] DMAs: the x-path is
    per-DMA-overhead-bound (512 DMAs/pass -> 195GB/s; 256 -> 354GB/s),
    so pairing k-tiles lifts the stream to ~HBM limit, 2.3x the PE's
    consumption rate.
"""

import os
import sys

sys.path.insert(0, "/opt/trn_rl_repo")

import numpy as np

B, S, D_IN, D_OUT, PALETTE = 4, 2048, 4096, 4096, 256
N_CORES = 8
M = B * S  # 8192
O_SHARD = D_OUT // N_CORES  # 512
P = 128
KO = D_IN // P  # 32 k-tiles
MG = M // 512  # 16 m-groups of 512 rows

# fp16 | bf16 | fp32r  (matmul input dtype; see module docstring)
MM_DTYPE = os.environ.get("KMEANS_MM_DTYPE", "fp16")
# >1 wraps the body in a device-side repeat loop (timing aid only)
REPEATS = int(os.environ.get("KMEANS_REPEATS", "1"))
# component isolation for timing experiments: all | xdma | mm | noepi
ONLY = os.environ.get("KMEANS_ONLY", "all")
# sharding strategy: o = column-parallel (x streamed), m = row-parallel
# (x resident, W streamed)
SHARD = os.environ.get("KMEANS_SHARD", "o")
M_SH = M // N_CORES  # 1024 rows per core (m-sharding)
OT = D_OUT // P  # 32 o-tiles (m-sharding)
# junk matmuls at kernel start: keeps PE busy while the first input DMAs
# land (~2us) and starts the HAM un-throttle window early.  Kept small —
# at the cold 1.2GHz clock each N=512 matmul is ~427ns, and overshooting
# the data-wait would delay the real stream.
WARM = int(os.environ.get("KMEANS_WARM", "4"))
# o-shard x DMA granularity: small = [128,512] per (mg,ko); big = [128,KO,512]
# per mg (4.2MB contiguous-ish, fewer ring transactions)
XMODE = os.environ.get("KMEANS_XMODE", "small")
# matmul issue order within an m-group: komi = rotate psum banks every MM;
# miko = 32 back-to-back accumulations per bank (needs XMODE=big or ONLY=mm)
XORDER = os.environ.get("KMEANS_XORDER", "komi")
# per-ko W-load split: W now streams on the ACT ring (x owns the SP ring),
# so the split no longer contends with x tiles and the first matmuls start
# after 1/32 of W instead of waiting out the full 4.2MB prologue
W_SPLIT = os.environ.get("KMEANS_W_SPLIT", "1") == "1"
# which HWDGE ring carries the y output DMAs: act keeps them off the
# x-stream ring
YQ = os.environ.get("KMEANS_YQ", "act")
# x DRAM layout: std = x^T [D_IN, M]; tiled = host pre-tiles to
# [(mg ko p), 512] so multi-ko x DMAs are possible.  Solo x-stream rate is
# per-DMA-overhead-bound: 512 DMAs/pass -> 195GB/s, 256 DMAs (XKO=2) ->
# 354GB/s (~HBM-per-NC limit), giving the stream 2.3x headroom over the
# 153.6GB/s the PE consumes.
XSRC = os.environ.get("KMEANS_XSRC", "tiled")
# k-tiles per x DMA (tiled layout only): 1 -> 131KB, 2 -> 262KB, 4 -> 524KB
XKO = int(os.environ.get("KMEANS_XKO", "2"))
X_BUFS = int(os.environ.get("KMEANS_X_BUFS", "12"))

_cache = {}


def _mm_dt():
    import concourse.mybir as mybir

    return {
        "fp16": (mybir.dt.float16, np.float16),
        "bf16": (mybir.dt.bfloat16, None),  # np side handled via ml_dtypes
        "fp32r": (mybir.dt.float32r, np.float32),
    }[MM_DTYPE]


def _np_cast(a):
    if MM_DTYPE == "fp16":
        return a.astype(np.float16)
    if MM_DTYPE == "bf16":
        import ml_dtypes

        return a.astype(ml_dtypes.bfloat16)
    return np.ascontiguousarray(a, dtype=np.float32)


def _emit_warmup(nc, warm_pool, pp, dt_mm):
    """Junk matmuls on a zeroed tile: occupy the PE through the HAM
    un-throttle window (~3.4us) while the first real DMAs land."""
    import concourse.mybir as mybir

    if WARM <= 0:
        return
    wx = warm_pool.tile([P, 512], dt_mm)
    nc.any.memset(wx[:], 0.0)
    wps = pp.tile([P, 512], mybir.dt.float32, tag="ps", name="warm_ps")
    for _ in range(WARM):
        nc.tensor.matmul(wps[:], wx[:, 0:P], wx[:], start=True, stop=True)


def _build():
    from concourse import bacc
    import concourse.mybir as mybir
    import concourse.tile as tile
    from concourse.bass import ds, ts

    dt_mm, _ = _mm_dt()
    nc = bacc.Bacc(None, target_bir_lowering=False)
    if XSRC == "tiled":
        xt = nc.dram_tensor("xt", [MG * KO * P, 512], dt_mm, kind="ExternalInput")
    else:
        xt = nc.dram_tensor("xt", [D_IN, M], dt_mm, kind="ExternalInput")
    wt = nc.dram_tensor("wt", [D_IN, O_SHARD], dt_mm, kind="ExternalInput")
    biasb = nc.dram_tensor("biasb", [P, O_SHARD], mybir.dt.float32, kind="ExternalInput")
    y = nc.dram_tensor("y", [M, O_SHARD], mybir.dt.float32, kind="ExternalOutput")

    with tile.TileContext(nc) as tc:
        with (
            tc.tile_pool(name="wpool", bufs=1) as wpool,
            tc.tile_pool(name="xpool", bufs=(4 if XMODE == "big" else X_BUFS)) as xpool,
            tc.tile_pool(name="opool", bufs=8) as opool,
            tc.tile_pool(name="cpool", bufs=1) as cpool,
            tc.tile_pool(name="warm", bufs=1) as warm_pool,
            tc.tile_pool(name="psum", bufs=8, space="PSUM") as pp,
        ):
            _emit_warmup(nc, warm_pool, pp, dt_mm)
            w_res = wpool.tile([P, KO, O_SHARD], dt_mm)
            wt_r = wt.rearrange("(ko p) o -> p ko o", p=P)
            # W on the ACT ring (x stream owns the SP ring); 2-ko chunks let
            # mg0's matmuls start after 1/16 of W while keeping per-DMA
            # overhead low enough to outpace mg0's consumption (154GB/s)
            if W_SPLIT:
                for kk in range(KO // 2):
                    nc.scalar.dma_start(
                        w_res[:, ds(2 * kk, 2), :], wt_r[:, ds(2 * kk, 2), :]
                    )
            else:
                nc.scalar.dma_start(w_res[:], wt_r)
            bias_t = cpool.tile([P, O_SHARD], mybir.dt.float32)
            nc.scalar.dma_start(bias_t[:], biasb[:])

            import contextlib

            rep_ctx = (
                tc.For_i(0, REPEATS, 1) if REPEATS > 1 else contextlib.nullcontext()
            )
            with rep_ctx:
                _emit_body(nc, tc, xpool, opool, pp, w_res, bias_t, xt, y)
    nc.compile()
    return nc


def _emit_body(nc, tc, xpool, opool, pp, w_res, bias_t, xt, y):
    import concourse.mybir as mybir
    from concourse.bass import ds, ts

    dt_mm, _ = _mm_dt()
    if ONLY == "xdma":
        # x-stream only: measures achievable x DMA bandwidth
        for mg in range(MG):
            if XSRC == "tiled":
                for kk in range(KO // XKO):
                    xt_t = xpool.tile([P, XKO, 512], dt_mm, tag="xt")
                    base = (mg * KO + kk * XKO) * P
                    nc.sync.dma_start(
                        xt_t[:],
                        xt[ds(base, XKO * P), :].rearrange("(g p) m -> p g m", p=P),
                    )
            else:
                for ko in range(KO):
                    xt_t = xpool.tile([P, 512], dt_mm, tag="xt")
                    nc.sync.dma_start(xt_t[:], xt[ds(ko * P, P), ds(mg * 512, 512)])
        return
    if ONLY == "xdma2q":
        # x-stream split across both HWDGE queues (SP + Activation)
        for mg in range(MG):
            for ko in range(KO):
                xt_t = xpool.tile([P, 512], dt_mm, tag="xt")
                eng = nc.sync if ko % 2 == 0 else nc.scalar
                eng.dma_start(xt_t[:], xt[ds(ko * P, P), ds(mg * 512, 512)])
        return
    if ONLY == "xdmabig":
        # x-stream as one 4.2MB DMA per m-group (16 per iteration)
        xt_r = xt.rearrange("(ko p) m -> p ko m", p=P)
        for mg in range(MG):
            xt_t = xpool.tile([P, KO, 512], dt_mm, tag="xb")
            nc.sync.dma_start(xt_t[:], xt_r[:, :, ds(mg * 512, 512)])
        return
    x_static = None
    if ONLY == "mm":
        # PE-only: single memset x tile reused by every matmul
        x_static = xpool.tile([P, 512], dt_mm, tag="xs")
        nc.any.memset(x_static[:], 0.0)
    xt_r = xt.rearrange("(ko p) m -> p ko m", p=P)
    if True:
            for mg in range(MG):
                psums = [
                    pp.tile([P, O_SHARD], mybir.dt.float32, tag="ps", name=f"ps_{mg}_{i}")
                    for i in range(4)
                ]
                xt_big = None
                if XMODE == "big" and ONLY != "mm":
                    xt_big = xpool.tile([P, KO, 512], dt_mm, tag="xb")
                    if mg == 0:
                        # chunked: first matmuls start after 1/32 of the tile
                        for ko in range(KO):
                            nc.sync.dma_start(
                                xt_big[:, ko, :], xt_r[:, ko, ds(mg * 512, 512)]
                            )
                    else:
                        nc.sync.dma_start(xt_big[:], xt_r[:, :, ds(mg * 512, 512)])
                if ONLY != "mm" and XMODE != "big" and XSRC == "tiled":
                    assert XORDER == "komi", "miko needs XMODE=big"
                    for kk in range(KO // XKO):
                        t = xpool.tile([P, XKO, 512], dt_mm, tag="xt")
                        base = (mg * KO + kk * XKO) * P
                        nc.sync.dma_start(
                            t[:],
                            xt[ds(base, XKO * P), :].rearrange(
                                "(g p) m -> p g m", p=P
                            ),
                        )
                        for j in range(XKO):
                            ko = kk * XKO + j
                            for mi in range(4):
                                nc.tensor.matmul(
                                    psums[mi][:], t[:, j, ts(mi, P)],
                                    w_res[:, ko, :],
                                    start=(ko == 0), stop=(ko == KO - 1),
                                )
                elif ONLY != "mm" and XMODE != "big":
                    assert XORDER == "komi", "miko needs XMODE=big"
                    for ko in range(KO):
                        t = xpool.tile([P, 512], dt_mm, tag="xt")
                        nc.sync.dma_start(t[:], xt[ds(ko * P, P), ds(mg * 512, 512)])
                        for mi in range(4):
                            nc.tensor.matmul(
                                psums[mi][:], t[:, ts(mi, P)], w_res[:, ko, :],
                                start=(ko == 0), stop=(ko == KO - 1),
                            )
                else:
                    if XORDER == "miko":
                        order = [(ko, mi) for mi in range(4) for ko in range(KO)]
                    else:
                        order = [(ko, mi) for ko in range(KO) for mi in range(4)]
                    for ko, mi in order:
                        lhsT = (
                            x_static[:, ts(mi, P)] if ONLY == "mm"
                            else xt_big[:, ko, ts(mi, P)]
                        )
                        nc.tensor.matmul(
                            psums[mi][:], lhsT, w_res[:, ko, :],
                            start=(ko == 0), stop=(ko == KO - 1),
                        )
                if ONLY in ("noepi", "mm"):
                    continue
                for mi in range(4):
                    ot = opool.tile([P, O_SHARD], mybir.dt.float32, tag="ot")
                    nc.vector.tensor_tensor(
                        ot[:], psums[mi][:], bias_t[:], mybir.AluOpType.add
                    )
                    # y out on the ACT HWDGE queue: keeps the compute-gated
                    # output DMA from blocking the x-stream prefetch on SP
                    yeng = nc.scalar if YQ == "act" else nc.sync
                    yeng.dma_start(y[ds(mg * 512 + mi * P, P), :], ot[:])


def _build_ms():
    """Row-parallel (M-sharded) build: x^T resident in SBUF, W^T streamed.

    Per core: Y_c[1024m, 4096o] = X_c[1024, 4096] @ W^T + bias.
    Loop over 32 o-tiles; per o-tile stream W^T chunk [4096k, 128o] (1 MB,
    contiguous), matmul against resident x^T with W-ktile stationary
    (reused for 2 m-groups), psum [128o, 512m], ACT adds per-partition
    bias, DMA out y^T-tiled.
    """
    from concourse import bacc
    import concourse.mybir as mybir
    import concourse.tile as tile
    from concourse.bass import ds, ts

    dt_mm, _ = _mm_dt()
    nc = bacc.Bacc(None, target_bir_lowering=False)
    # xq[ko*128+p, m] = x_c[m, ko*128+p]  (= x_c.T, contiguous)
    xq = nc.dram_tensor("xq", [D_IN, M_SH], dt_mm, kind="ExternalInput")
    # wq[ot*128+p, ko*128+o] = W^T[ko*128+p, ot*128+o]
    wq = nc.dram_tensor("wq", [OT * P, KO * P], dt_mm, kind="ExternalInput")
    # bq[p, ot] = bias[ot*128+p]
    bq = nc.dram_tensor("bq", [P, OT], mybir.dt.float32, kind="ExternalInput")
    # yq[(ot*2+mgr)*128+p, m] = y_c[mgr*512+m, ot*128+p]
    y = nc.dram_tensor("y", [OT * 2 * P, 512], mybir.dt.float32, kind="ExternalOutput")

    with tile.TileContext(nc) as tc:
        with (
            tc.tile_pool(name="xres", bufs=1) as xrp,
            tc.tile_pool(name="wpool", bufs=4) as wpool,
            tc.tile_pool(name="opool", bufs=6) as opool,
            tc.tile_pool(name="cpool", bufs=1) as cpool,
            tc.tile_pool(name="psum", bufs=8, space="PSUM") as pp,
        ):
            bias_sb = cpool.tile([P, OT], mybir.dt.float32)
            nc.sync.dma_start(bias_sb[:], bq[:])
            x_res = xrp.tile([P, KO, M_SH], dt_mm)
            for ko in range(KO):
                nc.sync.dma_start(x_res[:, ko, :], xq[ds(ko * P, P), :])

            import contextlib

            rep_ctx = (
                tc.For_i(0, REPEATS, 1) if REPEATS > 1 else contextlib.nullcontext()
            )
            with rep_ctx:
                for ot in range(OT):
                    w_t = wpool.tile([P, KO, P], dt_mm, tag="wt")
                    nc.sync.dma_start(
                        w_t[:], wq[ds(ot * P, P), :].rearrange("p (ko o) -> p ko o", o=P)
                    )
                    psums = [
                        pp.tile([P, 512], mybir.dt.float32, tag="ps", name=f"ps_{ot}_{i}")
                        for i in range(2)
                    ]
                    for ko in range(KO):
                        for mgr in range(2):
                            nc.tensor.matmul(
                                psums[mgr][:],
                                w_t[:, ko, :],
                                x_res[:, ko, ds(mgr * 512, 512)],
                                start=(ko == 0),
                                stop=(ko == KO - 1),
                            )
                    for mgr in range(2):
                        osb = opool.tile([P, 512], mybir.dt.float32, tag="ot")
                        nc.scalar.add(osb[:], psums[mgr][:], bias_sb[:, ds(ot, 1)])
                        # y out on ACT queue: SP carries only the W stream, so
                        # w(ot+1) issues as soon as its WAR clears instead of
                        # queueing behind this compute-gated output DMA
                        nc.scalar.dma_start(y[ds((ot * 2 + mgr) * P, P), :], osb[:])
    nc.compile()
    return nc


def make_in_maps_ms(input, lookup_table, weight_idx, bias):
    x = np.asarray(input, dtype=np.float32).reshape(M, D_IN)
    lut = np.asarray(lookup_table, dtype=np.float32)
    idx = np.asarray(weight_idx)
    b = np.asarray(bias, dtype=np.float32)

    wt_full = _np_cast(lut[idx].T)  # [D_IN, D_OUT]
    # wq[ot*128+p, ko*128+o] = wt_full[ko*128+p, ot*128+o]
    wq = np.ascontiguousarray(
        wt_full.reshape(KO, P, OT, P).transpose(2, 1, 0, 3).reshape(OT * P, KO * P)
    )
    bq = np.ascontiguousarray(b.reshape(OT, P).T)  # [P, OT]

    in_maps = []
    for c in range(N_CORES):
        xc = x[c * M_SH : (c + 1) * M_SH]  # [1024, 4096]
        xq = np.ascontiguousarray(_np_cast(xc).T)  # [D_IN, M_SH]
        in_maps.append({"xq": xq, "wq": wq, "bq": bq})
    return in_maps


def gather_ms(results):
    ys = []
    for c in range(N_CORES):
        yq = results[c]["y"].reshape(OT, 2, P, 512)
        ys.append(np.transpose(yq, (1, 3, 0, 2)).reshape(M_SH, D_OUT))
    return np.concatenate(ys, axis=0).reshape(B, S, D_OUT)


def get_nc():
    if "nc" not in _cache:
        _cache["nc"] = _build_ms() if SHARD == "m" else _build()
    return _cache["nc"]


def make_in_maps(input, lookup_table, weight_idx, bias):
    """Host-side shard/layout prep -> per-core input maps."""
    x = np.asarray(input, dtype=np.float32).reshape(M, D_IN)
    lut = np.asarray(lookup_table, dtype=np.float32)
    idx = np.asarray(weight_idx)
    b = np.asarray(bias, dtype=np.float32)

    xT = _np_cast(x).T  # [D_IN, M]
    if XSRC == "tiled":
        # pre-tile so each (mg, ko) x tile is one contiguous 131KB DRAM chunk
        xt = np.ascontiguousarray(
            xT.reshape(KO, P, MG, 512).transpose(2, 0, 1, 3).reshape(MG * KO * P, 512)
        )
    else:
        xt = np.ascontiguousarray(xT)
    wt_full = lut[idx].T  # [D_IN, D_OUT] f32 (palette dequant on host)

    in_maps = []
    for c in range(N_CORES):
        sl = slice(c * O_SHARD, (c + 1) * O_SHARD)
        in_maps.append(
            {
                "xt": xt,
                "wt": np.ascontiguousarray(_np_cast(wt_full[:, sl])),
                "biasb": np.ascontiguousarray(
                    np.broadcast_to(b[sl], (P, O_SHARD)), dtype=np.float32
                ),
            }
        )
    return in_maps


def kernel(input, lookup_table, weight_idx, bias):
    from concourse.bass_utils import run_bass_kernel_spmd

    nc = get_nc()
    if SHARD == "m":
        in_maps = make_in_maps_ms(input, lookup_table, weight_idx, bias)
        res = run_bass_kernel_spmd(nc, in_maps, core_ids=list(range(N_CORES)))
        return gather_ms(res.results)
    in_maps = make_in_maps(input, lookup_table, weight_idx, bias)
    res = run_bass_kernel_spmd(nc, in_maps, core_ids=list(range(N_CORES)))
    y = np.concatenate([res.results[c]["y"] for c in range(N_CORES)], axis=1)
    return y.reshape(B, S, D_OUT)

